# revision 56
# baseline (speedup 1.0000x reference)
"""Sparse-attention transformer block on 8 Trainium2 NeuronCores (Bass/Tile).

Sharding: 8 cores = 4 batches x 2 query-halves (SPMD, one program).
Each core processes T=1024 query tokens of one batch. Key/value tokens are
host-gathered per core: the core's own 1024 tokens plus the (static-mask)
summary tokens its queries attend outside that range, padded to SKV=1408.
All activations are feature-major ("transposed", [feature, token]) so every
matmul contracts along partitions with zero on-device transposes:

  xT -> LN1 (partition reduce via ones-matmul) -> hT
  kT = Wk hT, qT = Wq hT (feature-major); V = hT^T Wv^T (token-major)
  scoresT[s,q] = kT^T qT per head; p = exp(s) * mask01 (scores are small:
  no max subtraction needed); oT[d,q] = V'^T p with a ones column in V'
  giving the softmax denominator for free; normalize, Wo, residual, LN2,
  MLP (gelu-tanh), residual -> outT.

Matmuls run in bf16 (tolerance 2e-2 >> bf16 error).
"""

import numpy as np
import ml_dtypes
from contextlib import ExitStack

import concourse.bass as bass
import concourse.bacc as bacc
import concourse.tile as tile
from concourse import mybir
from concourse import bass_utils

B, S, E, H, D = 4, 2048, 1024, 16, 64
HID = 4 * E
T = 1024            # query tokens per core
SKV = 1408          # gathered kv tokens per core (11 chunks of 128)
NCH = SKV // 128    # 11 s-chunks
NE = E // 128       # 8 feature chunks
NHT = HID // 128    # 32 hidden chunks
N_CORES = 8
# Active s-chunk pairs per query group. For queries q < 512 (qg0), keys are
# causally <= 511 (chunks 0-3) plus gathered summary chunks 8-10; chunks 4-7
# are fully masked for every core (host asserts this).
PAIRS_QG = [
    [(0, 1), (2, 3), (8, 9), (10,)],
    [(0, 1), (2, 3), (4, 5), (6, 7), (8, 9), (10,)],
]
MASK_J0 = [0, len(PAIRS_QG[0])]           # mask tile base index per qg
N_MASK = len(PAIRS_QG[0]) + len(PAIRS_QG[1])
MASK_W = 1024
QGS = [slice(0, 512), slice(512, 1024)]
KVGS = [slice(0, 512), slice(512, 1024), slice(1024, 1408)]
BF = mybir.dt.bfloat16
F32 = mybir.dt.float32
AF = mybir.ActivationFunctionType
OP = mybir.AluOpType

_CACHE = {}
import os
SIM_SAFE_GELU = os.environ.get("BASS_SIM_SAFE_GELU") == "1"


# ---------------------------------------------------------------- device code

def _tln(tc, ctx, src, dst, g_ap, b_ap, W, groups, dram, ones, eps_row, tag,
         stat_pool=None):
    """Transposed layernorm: src [128, NE, W] bf16 -> dst [128, NE, W] bf16.

    Stats are partition reductions via ones-matmuls; mean/rstd rows get
    partition-broadcast back through a DRAM bounce. g_ap/b_ap: [128, NE]."""
    nc = tc.nc
    stat = stat_pool if stat_pool is not None else ctx.enter_context(
        tc.tile_pool(name=f"stat{tag}", bufs=1, space="PSUM"))
    tmp = ctx.enter_context(tc.tile_pool(name=f"tmp{tag}", bufs=2))
    rows = ctx.enter_context(tc.tile_pool(name=f"rows{tag}", bufs=1))
    bc = ctx.enter_context(tc.tile_pool(name=f"bc{tag}", bufs=1))

    stag = "ps" if stat_pool is not None else "ps_mean"
    stag2 = "ps" if stat_pool is not None else "ps_sq"
    ps_mean = stat.tile([1, W], F32, tag=stag)
    ps_sq = stat.tile([1, W], F32, tag=stag2)
    for gs in groups:
        for c in range(NE):
            nc.tensor.matmul(ps_mean[0:1, gs], ones[:], src[:, c, gs],
                             start=(c == 0), stop=(c == NE - 1),
                             skip_group_check=True)
    for c in range(NE):
        sq = tmp.tile([128, W], BF, tag="sq")
        nc.scalar.activation(sq[:], src[:, c, :], AF.Square)
        for gs in groups:
            nc.tensor.matmul(ps_sq[0:1, gs], ones[:], sq[:, gs],
                             start=(c == 0), stop=(c == NE - 1),
                             skip_group_check=True)

    # rows chain with two recycled slots (A: mean, B: scratch)
    mean_r = rows.tile([1, W], F32, tag="rowA")
    nc.scalar.mul(mean_r[:], ps_mean[0:1, :], 1.0 / E)
    msq_r = rows.tile([1, W], F32, tag="rowB")
    nc.vector.tensor_mul(msq_r[:], mean_r[:], mean_r[:])
    var_r = rows.tile([1, W], F32, tag="rowC")
    nc.vector.scalar_tensor_tensor(var_r[:], ps_sq[0:1, :], 1.0 / E, msq_r[:],
                                   op0=OP.mult, op1=OP.subtract)
    std_r = rows.tile([1, W], F32, tag="rowB")
    nc.scalar.activation(std_r[:], var_r[:], AF.Sqrt, bias=eps_row[:])
    rstd_r = rows.tile([1, W], F32, tag="rowC")
    nc.vector.reciprocal(rstd_r[:], std_r[:])
    mean_h = rows.tile([1, W], BF, tag="rowH1")
    nc.vector.tensor_copy(mean_h[:], mean_r[:])
    rstd_h = rows.tile([1, W], BF, tag="rowH2")
    nc.vector.tensor_copy(rstd_h[:], rstd_r[:])

    drows = dram.tile([2, W], BF, tag=f"drows{tag}")
    nc.sync.dma_start(drows[0:1, :], mean_h[:])
    nc.sync.dma_start(drows[1:2, :], rstd_h[:])
    mean_b = bc.tile([128, W], BF, tag="mean_b")
    rstd_b = bc.tile([128, W], BF, tag="rstd_b")
    nc.sync.dma_start(mean_b[:], drows[0:1, :].partition_broadcast(128))
    nc.sync.dma_start(rstd_b[:], drows[1:2, :].partition_broadcast(128))

    for c in range(NE):
        t0 = tmp.tile([128, W], F32, tag="t0")
        nc.vector.tensor_sub(t0[:], src[:, c, :], mean_b[:])
        nc.vector.tensor_mul(t0[:], t0[:], rstd_b[:])
        nc.vector.tensor_scalar(out=dst[:, c, :], in0=t0[:],
                                scalar1=g_ap[:, c:c + 1],
                                scalar2=b_ap[:, c:c + 1],
                                op0=OP.mult, op1=OP.add)


def _program(tc, ctx, outT, ins):
    nc = tc.nc
    (xkv, wqs, wks, wvs, wos, wfcs, wprjs, mask_d, gb, bfc_d, bprj_d) = ins

    const = ctx.enter_context(tc.tile_pool(name="const", bufs=1))
    dram = ctx.enter_context(tc.tile_pool(name="dram", bufs=1, space="DRAM"))

    ones = const.tile([128, 1], BF)
    nc.vector.memset(ones[:], 1.0)
    eps_row = const.tile([1, 1], F32)
    nc.vector.memset(eps_row[:], 1e-5)
    # gb cols: 0:8 ln1_g, 8:16 ln1_b, 16:24 ln2_g, 24:32 ln2_b,
    #          32:40 bq, 40:48 bk, 48:56 bv, 56:64 bo
    gb_sb = const.tile([128, 64], F32)
    nc.sync.dma_start(gb_sb[:], gb[:])
    bfc_sb = const.tile([128, 32], F32)
    nc.sync.dma_start(bfc_sb[:], bfc_d[:])
    bprj_sb = const.tile([128, 8], F32)
    nc.sync.dma_start(bprj_sb[:], bprj_d[:])

    # long-lived tiles first (pool releases must be LIFO)
    h2_pool = ctx.enter_context(tc.tile_pool(name="h2T", bufs=1))
    h2T = h2_pool.tile([128, NE, T], BF)
    g2_pool = ctx.enter_context(tc.tile_pool(name="g2T", bufs=1))
    g2T = g2_pool.tile([128, NE, T], BF)

    mask_pool = tc.alloc_tile_pool(name="maskp", bufs=1)
    mask_sb = mask_pool.tile([128, N_MASK, MASK_W], BF)
    for j in range(N_MASK):
        nc.sync.dma_start(mask_sb[:, j, :], mask_d[j])

    hT_pool = tc.alloc_tile_pool(name="hT", bufs=1)
    hT = hT_pool.tile([128, NE, SKV], BF)

    # ---- Phase 1: load x, LN1 -> hT ------------------------------------
    with ExitStack() as p1:
        xp = p1.enter_context(tc.tile_pool(name="xkv", bufs=1))
        x_sb = xp.tile([128, NE, SKV], BF)
        for c in range(NE):
            nc.sync.dma_start(x_sb[:, c, :], xkv[c])
        _tln(tc, p1, x_sb, hT, gb_sb[:, 0:8], gb_sb[:, 8:16], SKV, KVGS,
             dram, ones, eps_row, "1")

    # ---- Phase 2: QKV projections --------------------------------------
    kqv = tc.alloc_tile_pool(name="kqv", bufs=1)
    kT = kqv.tile([128, NE, SKV], BF)
    qT = kqv.tile([128, NE, T], BF)
    v_sb = kqv.tile([128, NCH, H * 65], BF)
    v4 = v_sb.rearrange("p t (h d) -> p t h d", d=65)
    nc.vector.memset(v4[:, :, :, 64:65], 1.0)

    ws2_pool = tc.alloc_tile_pool(name="ws2", bufs=3)
    attn_pool = tc.alloc_tile_pool(name="attnT", bufs=1)
    attnT = attn_pool.tile([128, NE, T], BF)
    wv_pool = tc.alloc_tile_pool(name="wvall", bufs=1)
    wv_all = wv_pool.tile([128, 16, 512], BF)
    for g2 in range(2):
        for c in range(NE):
            nc.sync.dma_start(wv_all[:, g2 * NE + c, :], wvs[g2, c])

    with ExitStack() as p2:
        pj = p2.enter_context(tc.tile_pool(name="pj2", bufs=4, space="PSUM"))
        for (wdram, dst, gsl, bcol) in ((wks, kT, KVGS, 40), (wqs, qT, QGS, 32)):
            for eo in range(NE):
                wt = ws2_pool.tile([128, NE, 128], BF, tag="w")
                nc.sync.dma_start(wt[:], wdram[eo])
                for gs in gsl:
                    n = gs.stop - gs.start
                    ps = pj.tile([128, 512], F32, tag="ps")
                    for c in range(NE):
                        nc.tensor.matmul(ps[:, :n], wt[:, c, :], hT[:, c, gs],
                                         start=(c == 0), stop=(c == NE - 1),
                                         skip_group_check=True)
                    nc.vector.tensor_scalar_add(
                        out=dst[:, eo, gs], in0=ps[:, :n],
                        scalar1=gb_sb[:, bcol + eo:bcol + eo + 1])
        # V token-major; e_out groups of 512 = 8 heads each
        for tt in range(NCH):
            for g2 in range(2):
                ps = pj.tile([128, 512], F32, tag="ps")
                for c in range(NE):
                    nc.tensor.matmul(ps[:], hT[:, c, tt * 128:(tt + 1) * 128],
                                     wv_all[:, g2 * NE + c, :],
                                     start=(c == 0), stop=(c == NE - 1),
                                     skip_group_check=True)
                nc.vector.tensor_copy(
                    out=v4[:, tt, g2 * 8:(g2 + 1) * 8, 0:64],
                    in_=ps[:].rearrange("p (h d) -> p h d", d=64))

    # ---- Phase 3: attention + Wo + LN2 (overlapped per query group) -----
    wv_pool.release()

    with ExitStack() as p3:
        pss = p3.enter_context(tc.tile_pool(name="psS", bufs=2, space="PSUM"))
        pso = p3.enter_context(tc.tile_pool(name="psO", bufs=2, space="PSUM"))
        pjw = p3.enter_context(tc.tile_pool(name="pjW", bufs=2, space="PSUM"))
        ptp = p3.enter_context(tc.tile_pool(name="pT", bufs=4))
        rcp = p3.enter_context(tc.tile_pool(name="rcp", bufs=2))
        bnc = p3.enter_context(tc.tile_pool(name="bnc", bufs=6, space="DRAM"))

        for qg in range(2):
            qs = QGS[qg]
            pairs = PAIRS_QG[qg]
            npair = len(pairs)
            for h in range(H):
                pt, po = h // 2, (h % 2) * 64
                ps_o = pso.tile([128, 512], F32, tag="o")
                for pi, pair in enumerate(pairs):
                    ps_s = pss.tile([128, MASK_W], F32, tag="s")
                    for k, c in enumerate(pair):
                        nc.tensor.matmul(
                            ps_s[:, k * 512:k * 512 + 512],
                            kT[po:po + 64, pt, c * 128:(c + 1) * 128],
                            qT[po:po + 64, pt, qs],
                            start=True, stop=True, skip_group_check=True)
                    w = len(pair) * 512
                    pT = ptp.tile([128, MASK_W], BF, tag="pT")
                    nc.scalar.activation(pT[:, :w], ps_s[:, :w], AF.Exp)
                    nc.vector.tensor_mul(pT[:, :w], pT[:, :w],
                                         mask_sb[:, MASK_J0[qg] + pi, :w])
                    for k, c in enumerate(pair):
                        nc.tensor.matmul(
                            ps_o[0:65, :],
                            v_sb[:, c, h * 65:h * 65 + 65],
                            pT[:, k * 512:k * 512 + 512],
                            start=(pi == 0 and k == 0),
                            stop=(pi == npair - 1 and k == len(pair) - 1),
                            skip_group_check=True)
                # normalize by the denominator row (ones column of V'):
                # bounce the raw row through DRAM to partition-broadcast it,
                # then a fast approximate reciprocal on the broadcast tile
                # (custom-DVE op requires SBUF input: PSUM reads misbehave).
                sden = rcp.tile([1, 512], F32, tag="sden")
                nc.scalar.copy(sden[:], ps_o[64:65, :])
                dden = bnc.tile([1, 512], F32, tag="dden")
                nc.sync.dma_start(dden[:], sden[:])
                denb = rcp.tile([64, 512], F32, tag="denb")
                nc.sync.dma_start(denb[:], dden[:].partition_broadcast(64))
                nc.vector.reciprocal_approx_fast(out=denb[:], in_=denb[:])
                nc.vector.tensor_mul(attnT[po:po + 64, pt, qs],
                                     ps_o[0:64, :], denb[:])
                # + bv (per-partition in feature-major layout)
                nc.vector.tensor_scalar_add(
                    out=attnT[po:po + 64, pt, qs],
                    in0=attnT[po:po + 64, pt, qs],
                    scalar1=gb_sb[po:po + 64, 48 + pt:48 + pt + 1])

            # Wo + residual for this query group (overlaps the other group's
            # attention on PE gaps), then LN2 -> g2T for this query group.
            for eo in range(NE):
                wt = ws2_pool.tile([128, NE, 128], BF, tag="w")
                nc.sync.dma_start(wt[:], wos[eo])
                ps = pjw.tile([128, 512], F32, tag="ps")
                for c in range(NE):
                    nc.tensor.matmul(ps[:], wt[:, c, :], attnT[:, c, qs],
                                     start=(c == 0), stop=(c == NE - 1),
                                     skip_group_check=True)
                nc.vector.scalar_tensor_tensor(
                    h2T[:, eo, qs], ps[:], gb_sb[:, 56 + eo:56 + eo + 1],
                    hT[:, eo, qs], op0=OP.add, op1=OP.add)
            _tln(tc, p3, h2T[:, :, qs], g2T[:, :, qs], gb_sb[:, 16:24],
                 gb_sb[:, 24:32], 512, [slice(0, 512)], dram, ones, eps_row,
                 f"2{qg}", stat_pool=pjw)

    attn_pool.release()
    ws2_pool.release()
    kqv.release()
    hT_pool.release()
    mask_pool.release()

    # ---- Phase 5: MLP + residual -> outT -------------------------------
    with ExitStack() as p6:
        ws = p6.enter_context(tc.tile_pool(name="ws6", bufs=3))
        wp = p6.enter_context(tc.tile_pool(name="wp6", bufs=3))
        pj = p6.enter_context(tc.tile_pool(name="pj6", bufs=4, space="PSUM"))
        up = p6.enter_context(tc.tile_pool(name="uT", bufs=1))
        op_ = p6.enter_context(tc.tile_pool(name="outp", bufs=3))
        uT = up.tile([128, NHT, T], BF, tag="uT")
        for ht in range(NHT):
            wt = ws.tile([128, NE, 128], BF, tag="w")
            nc.sync.dma_start(wt[:], wfcs[ht])
            for qg in range(2):
                qs = QGS[qg]
                ps = pj.tile([128, 512], F32, tag="ps")
                for c in range(NE):
                    nc.tensor.matmul(ps[:], wt[:, c, :], g2T[:, c, qs],
                                     start=(c == 0), stop=(c == NE - 1),
                                     skip_group_check=True)
                if not SIM_SAFE_GELU:
                    nc.scalar.activation(uT[:, ht, qs], ps[:],
                                         AF.Gelu_apprx_tanh,
                                         bias=bfc_sb[:, ht:ht + 1])
                else:
                    # new_gelu(u) = u * sigmoid(2c*(u + 0.044715 u^3))
                    u_sb = ws.tile([128, 512], BF, tag="gelu_u")
                    nc.vector.tensor_scalar_add(
                        out=u_sb[:], in0=ps[:],
                        scalar1=bfc_sb[:, ht:ht + 1])
                    t = ws.tile([128, 512], BF, tag="gelu_t")
                    nc.scalar.activation(t[:], u_sb[:], AF.Square)
                    z = ws.tile([128, 512], BF, tag="gelu_z")
                    nc.vector.scalar_tensor_tensor(
                        z[:], t[:], 0.044715, u_sb[:],
                        op0=OP.mult, op1=OP.mult)
                    nc.vector.tensor_add(z[:], z[:], u_sb[:])
                    g = ws.tile([128, 512], BF, tag="gelu_g")
                    nc.scalar.activation(
                        g[:], z[:], AF.Sigmoid,
                        scale=float(2.0 * np.sqrt(2.0 / np.pi)))
                    nc.vector.tensor_mul(uT[:, ht, qs], u_sb[:], g[:])
        for eo in range(NE):
            wt = wp.tile([128, NHT, 128], BF, tag="w")
            nc.sync.dma_start(wt[:], wprjs[eo])
            for qg in range(2):
                qs = QGS[qg]
                ps = pj.tile([128, 512], F32, tag="ps")
                for c in range(NHT):
                    nc.tensor.matmul(ps[:], wt[:, c, :], uT[:, c, qs],
                                     start=(c == 0), stop=(c == NHT - 1),
                                     skip_group_check=True)
                ot = op_.tile([128, 512], F32, tag="ot")
                nc.vector.scalar_tensor_tensor(
                    ot[:], ps[:], bprj_sb[:, eo:eo + 1], h2T[:, eo, qs],
                    op0=OP.add, op1=OP.add)
                nc.sync.dma_start(outT[eo][:, qs], ot[:])


def _build():
    if "nc" in _CACHE:
        return _CACHE["nc"]
    nc = bacc.Bacc("TRN2", target_bir_lowering=False, debug=False,
                   num_devices=N_CORES)

    def din(name, shape, dt=BF):
        return nc.dram_tensor(name, shape, dt, kind="ExternalInput").ap()

    ins = [
        din("xkv", [NE, 128, SKV]),
        din("wqs", [NE, 128, NE, 128]),
        din("wks", [NE, 128, NE, 128]),
        din("wvs", [2, NE, 128, 512]),
        din("wos", [NE, 128, NE, 128]),
        din("wfcs", [NHT, 128, NE, 128]),
        din("wprjs", [NE, 128, NHT, 128]),
        din("mask", [N_MASK, 128, MASK_W]),
        din("gb", [128, 64], F32),
        din("bfc", [128, 32], F32),
        din("bprj", [128, 8], F32),
    ]
    outT = nc.dram_tensor("outT", [NE, 128, T], F32, kind="ExternalOutput").ap()

    with tile.TileContext(nc) as tc:
        with ExitStack() as ctx:
            _program(tc, ctx, outT, ins)
    nc.compile()
    _CACHE["nc"] = nc
    return nc


# ------------------------------------------------------------------ host code

def _prep_shared(args):
    (ln1_g, ln1_b, ln2_g, ln2_b, wq, bq, wk, bk, wv, bv, wo, bo,
     w_fc, b_fc, w_proj, b_proj) = [np.asarray(a, np.float32) for a in args]
    bf = ml_dtypes.bfloat16

    def eo_layout(wT, nk, nm):  # wT [nk*128, nm*128] -> [nm, 128, nk, 128]
        return np.ascontiguousarray(
            wT.reshape(nk, 128, nm, 128).transpose(2, 1, 0, 3).astype(bf))

    def cols(v, n):  # [n*128] -> [128, n]
        return np.ascontiguousarray(
            np.asarray(v, np.float32).reshape(n, 128).T)

    scale = np.float32(D ** -0.5)
    d = {}
    d["wqs"] = eo_layout((wq * scale).T, NE, NE)
    d["wks"] = eo_layout(wk.T, NE, NE)
    d["wvs"] = np.ascontiguousarray(
        wv.T.reshape(NE, 128, 2, 512).transpose(2, 0, 1, 3).astype(bf))
    d["wos"] = eo_layout(wo.T, NE, NE)
    d["wfcs"] = eo_layout(w_fc.T, NE, NHT)
    d["wprjs"] = eo_layout(w_proj.T, NHT, NE)

    gb = np.zeros((128, 64), np.float32)
    gb[:, 0:8] = cols(ln1_g, 8)
    gb[:, 8:16] = cols(ln1_b, 8)
    gb[:, 16:24] = cols(ln2_g, 8)
    gb[:, 24:32] = cols(ln2_b, 8)
    gb[:, 32:40] = cols(bq * scale, 8)
    gb[:, 40:48] = cols(bk, 8)
    gb[:, 48:56] = cols(bv, 8)
    gb[:, 56:64] = cols(bo, 8)
    d["gb"] = gb
    d["bfc"] = cols(b_fc, 32)
    d["bprj"] = cols(b_proj, 8)
    return d


def _core_inputs(x, mask, shared, core):
    bf = ml_dtypes.bfloat16
    b, half = divmod(core, 2)
    qtok = np.arange(half * T, (half + 1) * T)
    need = np.where(mask[qtok].any(axis=0))[0]
    extra = np.setdiff1d(need, qtok)
    nreal = T + len(extra)
    assert nreal <= SKV, (core, nreal)
    kv = np.concatenate([qtok, extra,
                         np.zeros(SKV - nreal, np.int64)])

    xkvT = x[b].T[:, kv].astype(bf)  # [E, SKV]
    m = np.zeros((T, SKV), np.float32)
    m[:, :nreal] = mask[np.ix_(qtok, kv[:nreal])]
    mT = m.T  # [SKV, T]
    mtiles = np.zeros((N_MASK, 128, MASK_W), np.float32)
    for qg in range(2):
        active = {c for pair in PAIRS_QG[qg] for c in pair}
        skipped = [c for c in range(NCH) if c not in active]
        for c in skipped:
            assert not mT[c * 128:(c + 1) * 128,
                          qg * 512:(qg + 1) * 512].any(), (core, qg, c)
        for pi, pair in enumerate(PAIRS_QG[qg]):
            for k, c in enumerate(pair):
                mtiles[MASK_J0[qg] + pi, :, k * 512:(k + 1) * 512] = \
                    mT[c * 128:(c + 1) * 128, qg * 512:(qg + 1) * 512]

    im = dict(shared)
    im["xkv"] = np.ascontiguousarray(xkvT.reshape(NE, 128, SKV))
    im["mask"] = np.ascontiguousarray(mtiles.astype(bf))
    return im, qtok


def kernel(x, ln1_g, ln1_b, ln2_g, ln2_b, wq, bq, wk, bk, wv, bv, wo, bo,
           w_fc, b_fc, w_proj, b_proj, mask):
    x = np.asarray(x, np.float32)
    mask = np.asarray(mask, bool)

    nc = _build()
    shared = _prep_shared((ln1_g, ln1_b, ln2_g, ln2_b, wq, bq, wk, bk, wv,
                           bv, wo, bo, w_fc, b_fc, w_proj, b_proj))

    in_maps, qtoks = [], []
    for core in range(N_CORES):
        im, qtok = _core_inputs(x, mask, shared, core)
        in_maps.append(im)
        qtoks.append(qtok)

    br = bass_utils.run_bass_kernel_spmd(nc, in_maps,
                                         core_ids=list(range(N_CORES)))
    out = np.empty((B, S, E), np.float32)
    for core in range(N_CORES):
        b, half = divmod(core, 2)
        oT = br.results[core]["outT"]  # [NE, 128, T]
        out[b, qtoks[core], :] = oT.reshape(E, T).T
    return out


# revision 57
# speedup vs baseline: 1.0541x; 1.0541x over previous
"""Sparse-attention transformer block on 8 Trainium2 NeuronCores (Bass/Tile).

Sharding: 8 cores = 4 batches x 2 query-halves (SPMD, one program).
Each core processes T=1024 query tokens of one batch. Key/value tokens are
host-gathered per core: the core's own 1024 tokens plus the (static-mask)
summary tokens its queries attend outside that range, padded to SKV=1408.
All activations are feature-major ("transposed", [feature, token]) so every
matmul contracts along partitions with zero on-device transposes:

  xT -> LN1 (partition reduce via ones-matmul) -> hT
  kT = Wk hT, qT = Wq hT (feature-major); V = hT^T Wv^T (token-major)
  scoresT[s,q] = kT^T qT per head; p = exp(s) * mask01 (scores are small:
  no max subtraction needed); oT[d,q] = V'^T p with a ones column in V'
  giving the softmax denominator for free; normalize, Wo, residual, LN2,
  MLP (gelu-tanh), residual -> outT.

Matmuls run in bf16 (tolerance 2e-2 >> bf16 error).
"""

import numpy as np
import ml_dtypes
from contextlib import ExitStack

import concourse.bass as bass
import concourse.bacc as bacc
import concourse.tile as tile
from concourse import mybir
from concourse import bass_utils

B, S, E, H, D = 4, 2048, 1024, 16, 64
HID = 4 * E
T = 1024            # query tokens per core
SKV = 1408          # gathered kv tokens per core (11 chunks of 128)
NCH = SKV // 128    # 11 s-chunks
NE = E // 128       # 8 feature chunks
NHT = HID // 128    # 32 hidden chunks
N_CORES = 8
# Active s-chunk pairs per query group. For queries q < 512 (qg0), keys are
# causally <= 511 (chunks 0-3) plus gathered summary chunks 8-10; chunks 4-7
# are fully masked for every core (host asserts this).
PAIRS_QG = [
    [(0, 1), (2, 3), (8, 9), (10,)],
    [(0, 1), (2, 3), (4, 5), (6, 7), (8, 9), (10,)],
]
MASK_J0 = [0, len(PAIRS_QG[0])]           # mask tile base index per qg
N_MASK = len(PAIRS_QG[0]) + len(PAIRS_QG[1])
MASK_W = 1024
QGS = [slice(0, 512), slice(512, 1024)]
KVGS = [slice(0, 512), slice(512, 1024), slice(1024, 1408)]
BF = mybir.dt.bfloat16
F32 = mybir.dt.float32
AF = mybir.ActivationFunctionType
OP = mybir.AluOpType

_CACHE = {}
import os
SIM_SAFE_GELU = os.environ.get("BASS_SIM_SAFE_GELU") == "1"


# ---------------------------------------------------------------- device code

def _tln(tc, ctx, src, dst, g_ap, b_ap, W, groups, dram, ones, eps_row, tag,
         stat_pool=None):
    """Transposed layernorm: src [128, NE, W] bf16 -> dst [128, NE, W] bf16.

    Stats are partition reductions via ones-matmuls; mean/rstd rows get
    partition-broadcast back through a DRAM bounce. g_ap/b_ap: [128, NE]."""
    nc = tc.nc
    stat = stat_pool if stat_pool is not None else ctx.enter_context(
        tc.tile_pool(name=f"stat{tag}", bufs=1, space="PSUM"))
    tmp = ctx.enter_context(tc.tile_pool(name=f"tmp{tag}", bufs=2))
    rows = ctx.enter_context(tc.tile_pool(name=f"rows{tag}", bufs=1))
    bc = ctx.enter_context(tc.tile_pool(name=f"bc{tag}", bufs=1))

    stag = "ps" if stat_pool is not None else "ps_mean"
    stag2 = "ps" if stat_pool is not None else "ps_sq"
    ps_mean = stat.tile([1, W], F32, tag=stag)
    ps_sq = stat.tile([1, W], F32, tag=stag2)
    for gs in groups:
        for c in range(NE):
            nc.tensor.matmul(ps_mean[0:1, gs], ones[:], src[:, c, gs],
                             start=(c == 0), stop=(c == NE - 1),
                             skip_group_check=True)
    for c in range(NE):
        sq = tmp.tile([128, W], BF, tag="sq")
        nc.scalar.activation(sq[:], src[:, c, :], AF.Square)
        for gs in groups:
            nc.tensor.matmul(ps_sq[0:1, gs], ones[:], sq[:, gs],
                             start=(c == 0), stop=(c == NE - 1),
                             skip_group_check=True)

    # rows chain with two recycled slots (A: mean, B: scratch)
    mean_r = rows.tile([1, W], F32, tag="rowA")
    nc.scalar.mul(mean_r[:], ps_mean[0:1, :], 1.0 / E)
    msq_r = rows.tile([1, W], F32, tag="rowB")
    nc.vector.tensor_mul(msq_r[:], mean_r[:], mean_r[:])
    var_r = rows.tile([1, W], F32, tag="rowC")
    nc.vector.scalar_tensor_tensor(var_r[:], ps_sq[0:1, :], 1.0 / E, msq_r[:],
                                   op0=OP.mult, op1=OP.subtract)
    std_r = rows.tile([1, W], F32, tag="rowB")
    nc.scalar.activation(std_r[:], var_r[:], AF.Sqrt, bias=eps_row[:])
    rstd_r = rows.tile([1, W], F32, tag="rowC")
    nc.vector.reciprocal(rstd_r[:], std_r[:])
    mean_h = rows.tile([1, W], BF, tag="rowH1")
    nc.vector.tensor_copy(mean_h[:], mean_r[:])
    rstd_h = rows.tile([1, W], BF, tag="rowH2")
    nc.vector.tensor_copy(rstd_h[:], rstd_r[:])

    drows = dram.tile([2, W], BF, tag=f"drows{tag}")
    nc.sync.dma_start(drows[0:1, :], mean_h[:])
    nc.sync.dma_start(drows[1:2, :], rstd_h[:])
    mean_b = bc.tile([128, W], BF, tag="mean_b")
    rstd_b = bc.tile([128, W], BF, tag="rstd_b")
    nc.sync.dma_start(mean_b[:], drows[0:1, :].partition_broadcast(128))
    nc.sync.dma_start(rstd_b[:], drows[1:2, :].partition_broadcast(128))

    for c in range(NE):
        t0 = tmp.tile([128, W], F32, tag="t0")
        nc.vector.tensor_sub(t0[:], src[:, c, :], mean_b[:])
        nc.vector.tensor_mul(t0[:], t0[:], rstd_b[:])
        nc.vector.tensor_scalar(out=dst[:, c, :], in0=t0[:],
                                scalar1=g_ap[:, c:c + 1],
                                scalar2=b_ap[:, c:c + 1],
                                op0=OP.mult, op1=OP.add)


def _program(tc, ctx, outT, ins):
    nc = tc.nc
    (xkv, wqs, wks, wvs, wos, wfcs, wprjs, mask_d, gb, bfc_d, bprj_d) = ins

    const = ctx.enter_context(tc.tile_pool(name="const", bufs=1))
    dram = ctx.enter_context(tc.tile_pool(name="dram", bufs=1, space="DRAM"))

    ones = const.tile([128, 1], BF)
    nc.vector.memset(ones[:], 1.0)
    eps_row = const.tile([1, 1], F32)
    nc.vector.memset(eps_row[:], 1e-5)
    # gb cols: 0:8 ln1_g, 8:16 ln1_b, 16:24 ln2_g, 24:32 ln2_b,
    #          32:40 bq, 40:48 bk, 48:56 bv, 56:64 bo
    gb_sb = const.tile([128, 64], F32)
    nc.sync.dma_start(gb_sb[:], gb[:])
    bfc_sb = const.tile([128, 32], F32)
    nc.sync.dma_start(bfc_sb[:], bfc_d[:])
    bprj_sb = const.tile([128, 8], F32)
    nc.sync.dma_start(bprj_sb[:], bprj_d[:])

    # long-lived tiles first (pool releases must be LIFO)
    h2_pool = ctx.enter_context(tc.tile_pool(name="h2T", bufs=1))
    h2T = h2_pool.tile([128, NE, T], BF)
    g2_pool = ctx.enter_context(tc.tile_pool(name="g2T", bufs=1))
    g2T = g2_pool.tile([128, NE, T], BF)

    mask_pool = tc.alloc_tile_pool(name="maskp", bufs=1)
    mask_sb = mask_pool.tile([128, N_MASK, MASK_W], BF)
    for j in range(N_MASK):
        nc.sync.dma_start(mask_sb[:, j, :], mask_d[j])

    hT_pool = tc.alloc_tile_pool(name="hT", bufs=1)
    hT = hT_pool.tile([128, NE, SKV], BF)

    # ---- Phase 1: load x, LN1 -> hT ------------------------------------
    with ExitStack() as p1:
        xp = p1.enter_context(tc.tile_pool(name="xkv", bufs=1))
        x_sb = xp.tile([128, NE, SKV], BF)
        for c in range(NE):
            nc.sync.dma_start(x_sb[:, c, :], xkv[c])
        _tln(tc, p1, x_sb, hT, gb_sb[:, 0:8], gb_sb[:, 8:16], SKV, KVGS,
             dram, ones, eps_row, "1")

    # ---- Phase 2: QKV projections --------------------------------------
    kqv = tc.alloc_tile_pool(name="kqv", bufs=1)
    kT = kqv.tile([128, NE, SKV], BF)
    qT = kqv.tile([128, NE, T], BF)
    v_sb = kqv.tile([128, NCH, H * 65], BF)
    v4 = v_sb.rearrange("p t (h d) -> p t h d", d=65)
    nc.vector.memset(v4[:, :, :, 64:65], 1.0)

    ws2_pool = tc.alloc_tile_pool(name="ws2", bufs=3)
    attn_pool = tc.alloc_tile_pool(name="attnT", bufs=1)
    attnT = attn_pool.tile([128, NE, T], BF)
    wv_pool = tc.alloc_tile_pool(name="wvall", bufs=1)
    wv_all = wv_pool.tile([128, 16, 512], BF)
    for g2 in range(2):
        for c in range(NE):
            nc.sync.dma_start(wv_all[:, g2 * NE + c, :], wvs[g2, c])

    with ExitStack() as p2:
        pj = p2.enter_context(tc.tile_pool(name="pj2", bufs=4, space="PSUM"))
        for (wdram, dst, gsl, bcol) in ((wks, kT, KVGS, 40), (wqs, qT, QGS, 32)):
            for eo in range(NE):
                wt = ws2_pool.tile([128, NE, 128], BF, tag="w")
                nc.sync.dma_start(wt[:], wdram[eo])
                for gs in gsl:
                    n = gs.stop - gs.start
                    ps = pj.tile([128, 512], F32, tag="ps")
                    for c in range(NE):
                        nc.tensor.matmul(ps[:, :n], wt[:, c, :], hT[:, c, gs],
                                         start=(c == 0), stop=(c == NE - 1),
                                         skip_group_check=True)
                    nc.vector.tensor_scalar_add(
                        out=dst[:, eo, gs], in0=ps[:, :n],
                        scalar1=gb_sb[:, bcol + eo:bcol + eo + 1])
        # V token-major; e_out groups of 512 = 8 heads each
        for tt in range(NCH):
            for g2 in range(2):
                ps = pj.tile([128, 512], F32, tag="ps")
                for c in range(NE):
                    nc.tensor.matmul(ps[:], hT[:, c, tt * 128:(tt + 1) * 128],
                                     wv_all[:, g2 * NE + c, :],
                                     start=(c == 0), stop=(c == NE - 1),
                                     skip_group_check=True)
                nc.vector.tensor_copy(
                    out=v4[:, tt, g2 * 8:(g2 + 1) * 8, 0:64],
                    in_=ps[:].rearrange("p (h d) -> p h d", d=64))

    # ---- Phase 3: attention ---------------------------------------------
    wv_pool.release()

    with ExitStack() as p3:
        pss = p3.enter_context(tc.tile_pool(name="psS", bufs=2, space="PSUM"))
        pso = p3.enter_context(tc.tile_pool(name="psO", bufs=3, space="PSUM"))
        ptp = p3.enter_context(tc.tile_pool(name="pT", bufs=4))
        rcp = p3.enter_context(tc.tile_pool(name="rcp", bufs=4))
        bnc = p3.enter_context(tc.tile_pool(name="bnc", bufs=6, space="DRAM"))

        for qg in range(2):
            qs = QGS[qg]
            pairs = PAIRS_QG[qg]
            npair = len(pairs)
            for h in range(H):
                pt, po = h // 2, (h % 2) * 64
                ps_o = pso.tile([128, 512], F32, tag="o")
                for pi, pair in enumerate(pairs):
                    ps_s = pss.tile([128, MASK_W], F32, tag="s")
                    for k, c in enumerate(pair):
                        nc.tensor.matmul(
                            ps_s[:, k * 512:k * 512 + 512],
                            kT[po:po + 64, pt, c * 128:(c + 1) * 128],
                            qT[po:po + 64, pt, qs],
                            start=True, stop=True, skip_group_check=True)
                    w = len(pair) * 512
                    pT = ptp.tile([128, MASK_W], BF, tag="pT")
                    nc.scalar.activation(pT[:, :w], ps_s[:, :w], AF.Exp)
                    nc.vector.tensor_mul(pT[:, :w], pT[:, :w],
                                         mask_sb[:, MASK_J0[qg] + pi, :w])
                    for k, c in enumerate(pair):
                        nc.tensor.matmul(
                            ps_o[0:65, :],
                            v_sb[:, c, h * 65:h * 65 + 65],
                            pT[:, k * 512:k * 512 + 512],
                            start=(pi == 0 and k == 0),
                            stop=(pi == npair - 1 and k == len(pair) - 1),
                            skip_group_check=True)
                # normalize by the denominator row (ones column of V'):
                # bounce the raw row through DRAM to partition-broadcast it,
                # then a fast approximate reciprocal on the broadcast tile
                # (custom-DVE op requires SBUF input: PSUM reads misbehave).
                sden = rcp.tile([1, 512], F32, tag="sden")
                nc.vector.tensor_copy(sden[:], ps_o[64:65, :])
                dden = bnc.tile([1, 512], F32, tag="dden")
                nc.sync.dma_start(dden[:], sden[:])
                denb = rcp.tile([64, 512], F32, tag="denb")
                nc.sync.dma_start(denb[:], dden[:].partition_broadcast(64))
                nc.vector.reciprocal_approx_fast(out=denb[:], in_=denb[:])
                nc.vector.tensor_mul(attnT[po:po + 64, pt, qs],
                                     ps_o[0:64, :], denb[:])
                # + bv (per-partition in feature-major layout)
                nc.vector.tensor_scalar_add(
                    out=attnT[po:po + 64, pt, qs],
                    in0=attnT[po:po + 64, pt, qs],
                    scalar1=gb_sb[po:po + 64, 48 + pt:48 + pt + 1])

    # ---- Phase 4: Wo + residual -> h2T; LN2 -> g2T ---------------------
    with ExitStack() as p4:
        pj = p4.enter_context(tc.tile_pool(name="pj4", bufs=4, space="PSUM"))
        for qg in range(2):
            qs = QGS[qg]
            for eo in range(NE):
                wt = ws2_pool.tile([128, NE, 128], BF, tag="w")
                nc.sync.dma_start(wt[:], wos[eo])
                ps = pj.tile([128, 512], F32, tag="ps")
                for c in range(NE):
                    nc.tensor.matmul(ps[:], wt[:, c, :], attnT[:, c, qs],
                                     start=(c == 0), stop=(c == NE - 1),
                                     skip_group_check=True)
                nc.vector.scalar_tensor_tensor(
                    h2T[:, eo, qs], ps[:], gb_sb[:, 56 + eo:56 + eo + 1],
                    hT[:, eo, qs], op0=OP.add, op1=OP.add)
            _tln(tc, p4, h2T[:, :, qs], g2T[:, :, qs], gb_sb[:, 16:24],
                 gb_sb[:, 24:32], 512, [slice(0, 512)], dram, ones, eps_row,
                 f"2{qg}")

    attn_pool.release()
    ws2_pool.release()
    kqv.release()
    hT_pool.release()
    mask_pool.release()

    # ---- Phase 5: MLP + residual -> outT -------------------------------
    with ExitStack() as p6:
        ws = p6.enter_context(tc.tile_pool(name="ws6", bufs=3))
        wp = p6.enter_context(tc.tile_pool(name="wp6", bufs=3))
        pj = p6.enter_context(tc.tile_pool(name="pj6", bufs=4, space="PSUM"))
        up = p6.enter_context(tc.tile_pool(name="uT", bufs=1))
        op_ = p6.enter_context(tc.tile_pool(name="outp", bufs=3))
        uT = up.tile([128, NHT, T], BF, tag="uT")
        for ht in range(NHT):
            wt = ws.tile([128, NE, 128], BF, tag="w")
            nc.sync.dma_start(wt[:], wfcs[ht])
            for qg in range(2):
                qs = QGS[qg]
                ps = pj.tile([128, 512], F32, tag="ps")
                for c in range(NE):
                    nc.tensor.matmul(ps[:], wt[:, c, :], g2T[:, c, qs],
                                     start=(c == 0), stop=(c == NE - 1),
                                     skip_group_check=True)
                if not SIM_SAFE_GELU:
                    nc.scalar.activation(uT[:, ht, qs], ps[:],
                                         AF.Gelu_apprx_tanh,
                                         bias=bfc_sb[:, ht:ht + 1])
                else:
                    # new_gelu(u) = u * sigmoid(2c*(u + 0.044715 u^3))
                    u_sb = ws.tile([128, 512], BF, tag="gelu_u")
                    nc.vector.tensor_scalar_add(
                        out=u_sb[:], in0=ps[:],
                        scalar1=bfc_sb[:, ht:ht + 1])
                    t = ws.tile([128, 512], BF, tag="gelu_t")
                    nc.scalar.activation(t[:], u_sb[:], AF.Square)
                    z = ws.tile([128, 512], BF, tag="gelu_z")
                    nc.vector.scalar_tensor_tensor(
                        z[:], t[:], 0.044715, u_sb[:],
                        op0=OP.mult, op1=OP.mult)
                    nc.vector.tensor_add(z[:], z[:], u_sb[:])
                    g = ws.tile([128, 512], BF, tag="gelu_g")
                    nc.scalar.activation(
                        g[:], z[:], AF.Sigmoid,
                        scale=float(2.0 * np.sqrt(2.0 / np.pi)))
                    nc.vector.tensor_mul(uT[:, ht, qs], u_sb[:], g[:])
        for eo in range(NE):
            wt = wp.tile([128, NHT, 128], BF, tag="w")
            nc.sync.dma_start(wt[:], wprjs[eo])
            for qg in range(2):
                qs = QGS[qg]
                ps = pj.tile([128, 512], F32, tag="ps")
                for c in range(NHT):
                    nc.tensor.matmul(ps[:], wt[:, c, :], uT[:, c, qs],
                                     start=(c == 0), stop=(c == NHT - 1),
                                     skip_group_check=True)
                ot = op_.tile([128, 512], F32, tag="ot")
                nc.vector.scalar_tensor_tensor(
                    ot[:], ps[:], bprj_sb[:, eo:eo + 1], h2T[:, eo, qs],
                    op0=OP.add, op1=OP.add)
                nc.sync.dma_start(outT[eo][:, qs], ot[:])


def _build():
    if "nc" in _CACHE:
        return _CACHE["nc"]
    nc = bacc.Bacc("TRN2", target_bir_lowering=False, debug=False,
                   num_devices=N_CORES)

    def din(name, shape, dt=BF):
        return nc.dram_tensor(name, shape, dt, kind="ExternalInput").ap()

    ins = [
        din("xkv", [NE, 128, SKV]),
        din("wqs", [NE, 128, NE, 128]),
        din("wks", [NE, 128, NE, 128]),
        din("wvs", [2, NE, 128, 512]),
        din("wos", [NE, 128, NE, 128]),
        din("wfcs", [NHT, 128, NE, 128]),
        din("wprjs", [NE, 128, NHT, 128]),
        din("mask", [N_MASK, 128, MASK_W]),
        din("gb", [128, 64], F32),
        din("bfc", [128, 32], F32),
        din("bprj", [128, 8], F32),
    ]
    outT = nc.dram_tensor("outT", [NE, 128, T], F32, kind="ExternalOutput").ap()

    with tile.TileContext(nc) as tc:
        with ExitStack() as ctx:
            _program(tc, ctx, outT, ins)
    nc.compile()
    _CACHE["nc"] = nc
    return nc


# ------------------------------------------------------------------ host code

def _prep_shared(args):
    (ln1_g, ln1_b, ln2_g, ln2_b, wq, bq, wk, bk, wv, bv, wo, bo,
     w_fc, b_fc, w_proj, b_proj) = [np.asarray(a, np.float32) for a in args]
    bf = ml_dtypes.bfloat16

    def eo_layout(wT, nk, nm):  # wT [nk*128, nm*128] -> [nm, 128, nk, 128]
        return np.ascontiguousarray(
            wT.reshape(nk, 128, nm, 128).transpose(2, 1, 0, 3).astype(bf))

    def cols(v, n):  # [n*128] -> [128, n]
        return np.ascontiguousarray(
            np.asarray(v, np.float32).reshape(n, 128).T)

    scale = np.float32(D ** -0.5)
    d = {}
    d["wqs"] = eo_layout((wq * scale).T, NE, NE)
    d["wks"] = eo_layout(wk.T, NE, NE)
    d["wvs"] = np.ascontiguousarray(
        wv.T.reshape(NE, 128, 2, 512).transpose(2, 0, 1, 3).astype(bf))
    d["wos"] = eo_layout(wo.T, NE, NE)
    d["wfcs"] = eo_layout(w_fc.T, NE, NHT)
    d["wprjs"] = eo_layout(w_proj.T, NHT, NE)

    gb = np.zeros((128, 64), np.float32)
    gb[:, 0:8] = cols(ln1_g, 8)
    gb[:, 8:16] = cols(ln1_b, 8)
    gb[:, 16:24] = cols(ln2_g, 8)
    gb[:, 24:32] = cols(ln2_b, 8)
    gb[:, 32:40] = cols(bq * scale, 8)
    gb[:, 40:48] = cols(bk, 8)
    gb[:, 48:56] = cols(bv, 8)
    gb[:, 56:64] = cols(bo, 8)
    d["gb"] = gb
    d["bfc"] = cols(b_fc, 32)
    d["bprj"] = cols(b_proj, 8)
    return d


def _core_inputs(x, mask, shared, core):
    bf = ml_dtypes.bfloat16
    b, half = divmod(core, 2)
    qtok = np.arange(half * T, (half + 1) * T)
    need = np.where(mask[qtok].any(axis=0))[0]
    extra = np.setdiff1d(need, qtok)
    nreal = T + len(extra)
    assert nreal <= SKV, (core, nreal)
    kv = np.concatenate([qtok, extra,
                         np.zeros(SKV - nreal, np.int64)])

    xkvT = x[b].T[:, kv].astype(bf)  # [E, SKV]
    m = np.zeros((T, SKV), np.float32)
    m[:, :nreal] = mask[np.ix_(qtok, kv[:nreal])]
    mT = m.T  # [SKV, T]
    mtiles = np.zeros((N_MASK, 128, MASK_W), np.float32)
    for qg in range(2):
        active = {c for pair in PAIRS_QG[qg] for c in pair}
        skipped = [c for c in range(NCH) if c not in active]
        for c in skipped:
            assert not mT[c * 128:(c + 1) * 128,
                          qg * 512:(qg + 1) * 512].any(), (core, qg, c)
        for pi, pair in enumerate(PAIRS_QG[qg]):
            for k, c in enumerate(pair):
                mtiles[MASK_J0[qg] + pi, :, k * 512:(k + 1) * 512] = \
                    mT[c * 128:(c + 1) * 128, qg * 512:(qg + 1) * 512]

    im = dict(shared)
    im["xkv"] = np.ascontiguousarray(xkvT.reshape(NE, 128, SKV))
    im["mask"] = np.ascontiguousarray(mtiles.astype(bf))
    return im, qtok


def kernel(x, ln1_g, ln1_b, ln2_g, ln2_b, wq, bq, wk, bk, wv, bv, wo, bo,
           w_fc, b_fc, w_proj, b_proj, mask):
    x = np.asarray(x, np.float32)
    mask = np.asarray(mask, bool)

    nc = _build()
    shared = _prep_shared((ln1_g, ln1_b, ln2_g, ln2_b, wq, bq, wk, bk, wv,
                           bv, wo, bo, w_fc, b_fc, w_proj, b_proj))

    in_maps, qtoks = [], []
    for core in range(N_CORES):
        im, qtok = _core_inputs(x, mask, shared, core)
        in_maps.append(im)
        qtoks.append(qtok)

    br = bass_utils.run_bass_kernel_spmd(nc, in_maps,
                                         core_ids=list(range(N_CORES)))
    out = np.empty((B, S, E), np.float32)
    for core in range(N_CORES):
        b, half = divmod(core, 2)
        oT = br.results[core]["outT"]  # [NE, 128, T]
        out[b, qtoks[core], :] = oT.reshape(E, T).T
    return out


# revision 59
# speedup vs baseline: 1.0806x; 1.0252x over previous
"""Sparse-attention transformer block on 8 Trainium2 NeuronCores (Bass/Tile).

Sharding: 8 cores = 4 batches x 2 query-halves (SPMD, one program).
Each core processes T=1024 query tokens of one batch. Key/value tokens are
host-gathered per core: the core's own 1024 tokens plus the (static-mask)
summary tokens its queries attend outside that range, padded to SKV=1408.
All activations are feature-major ("transposed", [feature, token]) so every
matmul contracts along partitions with zero on-device transposes:

  xT -> LN1 (partition reduce via ones-matmul) -> hT
  kT = Wk hT, qT = Wq hT (feature-major); V = hT^T Wv^T (token-major)
  scoresT[s,q] = kT^T qT per head; p = exp(s) * mask01 (scores are small:
  no max subtraction needed); oT[d,q] = V'^T p with a ones column in V'
  giving the softmax denominator for free; normalize, Wo, residual, LN2,
  MLP (gelu-tanh), residual -> outT.

Matmuls run in bf16 (tolerance 2e-2 >> bf16 error).
"""

import numpy as np
import ml_dtypes
from contextlib import ExitStack

import concourse.bass as bass
import concourse.bacc as bacc
import concourse.tile as tile
from concourse import mybir
from concourse import bass_utils

B, S, E, H, D = 4, 2048, 1024, 16, 64
HID = 4 * E
T = 1024            # query tokens per core
SKV = 1408          # gathered kv tokens per core (11 chunks of 128)
NCH = SKV // 128    # 11 s-chunks
NE = E // 128       # 8 feature chunks
NHT = HID // 128    # 32 hidden chunks
N_CORES = 8
# Active s-chunk pairs per query group. For queries q < 512 (qg0), keys are
# causally <= 511 (chunks 0-3) plus gathered summary chunks 8-10; chunks 4-7
# are fully masked for every core (host asserts this).
PAIRS_QG = [
    [(0, 1), (2, 3), (8, 9), (10,)],
    [(0, 1), (2, 3), (4, 5), (6, 7), (8, 9), (10,)],
]
MASK_J0 = [0, len(PAIRS_QG[0])]           # mask tile base index per qg
N_MASK = len(PAIRS_QG[0]) + len(PAIRS_QG[1])
MASK_W = 1024
QGS = [slice(0, 512), slice(512, 1024)]
KVGS = [slice(0, 512), slice(512, 1024), slice(1024, 1408)]
BF = mybir.dt.bfloat16
F32 = mybir.dt.float32
AF = mybir.ActivationFunctionType
OP = mybir.AluOpType

_CACHE = {}
import os
SIM_SAFE_GELU = os.environ.get("BASS_SIM_SAFE_GELU") == "1"


# ---------------------------------------------------------------- device code

def _tln(tc, ctx, src, dst, g_ap, b_ap, W, groups, dram, ones, eps_row, tag,
         stat_pool=None):
    """Transposed layernorm: src [128, NE, W] bf16 -> dst [128, NE, W] bf16.

    Stats are partition reductions via ones-matmuls; mean/rstd rows get
    partition-broadcast back through a DRAM bounce. g_ap/b_ap: [128, NE]."""
    nc = tc.nc
    stat = stat_pool if stat_pool is not None else ctx.enter_context(
        tc.tile_pool(name=f"stat{tag}", bufs=1, space="PSUM"))
    tmp = ctx.enter_context(tc.tile_pool(name=f"tmp{tag}", bufs=2))
    rows = ctx.enter_context(tc.tile_pool(name=f"rows{tag}", bufs=1))
    bc = ctx.enter_context(tc.tile_pool(name=f"bc{tag}", bufs=1))

    stag = "ps" if stat_pool is not None else "ps_mean"
    stag2 = "ps" if stat_pool is not None else "ps_sq"
    ps_mean = stat.tile([1, W], F32, tag=stag)
    ps_sq = stat.tile([1, W], F32, tag=stag2)
    for gs in groups:
        for c in range(NE):
            nc.tensor.matmul(ps_mean[0:1, gs], ones[:], src[:, c, gs],
                             start=(c == 0), stop=(c == NE - 1),
                             skip_group_check=True)
    for c in range(NE):
        sq = tmp.tile([128, W], BF, tag="sq")
        nc.scalar.activation(sq[:], src[:, c, :], AF.Square)
        for gs in groups:
            nc.tensor.matmul(ps_sq[0:1, gs], ones[:], sq[:, gs],
                             start=(c == 0), stop=(c == NE - 1),
                             skip_group_check=True)

    # rows chain with two recycled slots (A: mean, B: scratch)
    mean_r = rows.tile([1, W], F32, tag="rowA")
    nc.scalar.mul(mean_r[:], ps_mean[0:1, :], 1.0 / E)
    msq_r = rows.tile([1, W], F32, tag="rowB")
    nc.vector.tensor_mul(msq_r[:], mean_r[:], mean_r[:])
    var_r = rows.tile([1, W], F32, tag="rowC")
    nc.vector.scalar_tensor_tensor(var_r[:], ps_sq[0:1, :], 1.0 / E, msq_r[:],
                                   op0=OP.mult, op1=OP.subtract)
    std_r = rows.tile([1, W], F32, tag="rowB")
    nc.scalar.activation(std_r[:], var_r[:], AF.Sqrt, bias=eps_row[:])
    rstd_r = rows.tile([1, W], F32, tag="rowC")
    nc.vector.reciprocal(rstd_r[:], std_r[:])
    mean_h = rows.tile([1, W], BF, tag="rowH1")
    nc.vector.tensor_copy(mean_h[:], mean_r[:])
    rstd_h = rows.tile([1, W], BF, tag="rowH2")
    nc.vector.tensor_copy(rstd_h[:], rstd_r[:])

    drows = dram.tile([2, W], BF, tag=f"drows{tag}")
    nc.sync.dma_start(drows[0:1, :], mean_h[:])
    nc.sync.dma_start(drows[1:2, :], rstd_h[:])
    mean_b = bc.tile([128, W], BF, tag="mean_b")
    rstd_b = bc.tile([128, W], BF, tag="rstd_b")
    nc.sync.dma_start(mean_b[:], drows[0:1, :].partition_broadcast(128))
    nc.sync.dma_start(rstd_b[:], drows[1:2, :].partition_broadcast(128))

    for c in range(NE):
        t0 = tmp.tile([128, W], F32, tag="t0")
        nc.vector.tensor_sub(t0[:], src[:, c, :], mean_b[:])
        nc.vector.tensor_mul(t0[:], t0[:], rstd_b[:])
        nc.vector.tensor_scalar(out=dst[:, c, :], in0=t0[:],
                                scalar1=g_ap[:, c:c + 1],
                                scalar2=b_ap[:, c:c + 1],
                                op0=OP.mult, op1=OP.add)


def _program(tc, ctx, outT, ins):
    nc = tc.nc
    (xkv, wqs, wks, wvs, wos, wfcs, wprjs, mask_d, gb, bfc_d, bprj_d) = ins

    const = ctx.enter_context(tc.tile_pool(name="const", bufs=1))
    dram = ctx.enter_context(tc.tile_pool(name="dram", bufs=1, space="DRAM"))

    ones = const.tile([128, 1], BF)
    nc.vector.memset(ones[:], 1.0)
    eps_row = const.tile([1, 1], F32)
    nc.vector.memset(eps_row[:], 1e-5)
    # gb cols: 0:8 ln1_g, 8:16 ln1_b, 16:24 ln2_g, 24:32 ln2_b,
    #          32:40 bq, 40:48 bk, 48:56 bv, 56:64 bo
    gb_sb = const.tile([128, 64], F32)
    nc.sync.dma_start(gb_sb[:], gb[:])
    bfc_sb = const.tile([128, 32], F32)
    nc.sync.dma_start(bfc_sb[:], bfc_d[:])
    bprj_sb = const.tile([128, 8], F32)
    nc.sync.dma_start(bprj_sb[:], bprj_d[:])

    # long-lived tiles first (pool releases must be LIFO)
    h2_pool = ctx.enter_context(tc.tile_pool(name="h2T", bufs=1))
    h2T = h2_pool.tile([128, NE, T], BF)
    g2_pool = ctx.enter_context(tc.tile_pool(name="g2T", bufs=1))
    g2T = g2_pool.tile([128, NE, T], BF)

    mask_pool = tc.alloc_tile_pool(name="maskp", bufs=1)
    mask_sb = mask_pool.tile([128, N_MASK, MASK_W], BF)
    for j in range(N_MASK):
        nc.sync.dma_start(mask_sb[:, j, :], mask_d[j])

    hT_pool = tc.alloc_tile_pool(name="hT", bufs=1)
    hT = hT_pool.tile([128, NE, SKV], BF)

    # ---- Phase 1: load x, LN1 -> hT ------------------------------------
    with ExitStack() as p1:
        xp = p1.enter_context(tc.tile_pool(name="xkv", bufs=1))
        x_sb = xp.tile([128, NE, SKV], BF)
        for c in range(NE):
            nc.sync.dma_start(x_sb[:, c, :], xkv[c])
        _tln(tc, p1, x_sb, hT, gb_sb[:, 0:8], gb_sb[:, 8:16], SKV, KVGS,
             dram, ones, eps_row, "1")

    # ---- Phase 2: QKV projections --------------------------------------
    kqv = tc.alloc_tile_pool(name="kqv", bufs=1)
    kT = kqv.tile([128, NE, SKV], BF)
    qT = kqv.tile([128, NE, T], BF)
    v_sb = kqv.tile([128, NCH, H * 65], BF)
    v4 = v_sb.rearrange("p t (h d) -> p t h d", d=65)
    nc.vector.memset(v4[:, :, :, 64:65], 1.0)

    ws2_pool = tc.alloc_tile_pool(name="ws2", bufs=3)
    attn_pool = tc.alloc_tile_pool(name="attnT", bufs=1)
    attnT = attn_pool.tile([128, NE, T], BF)
    wv_pool = tc.alloc_tile_pool(name="wvall", bufs=1)
    wv_all = wv_pool.tile([128, 16, 512], BF)
    for g2 in range(2):
        for c in range(NE):
            nc.sync.dma_start(wv_all[:, g2 * NE + c, :], wvs[g2, c])

    with ExitStack() as p2:
        pj = p2.enter_context(tc.tile_pool(name="pj2", bufs=4, space="PSUM"))
        for (wdram, dst, gsl, bcol) in ((wks, kT, KVGS, 40), (wqs, qT, QGS, 32)):
            for eo in range(NE):
                wt = ws2_pool.tile([128, NE, 128], BF, tag="w")
                nc.sync.dma_start(wt[:], wdram[eo])
                for gs in gsl:
                    n = gs.stop - gs.start
                    ps = pj.tile([128, 512], F32, tag="ps")
                    for c in range(NE):
                        nc.tensor.matmul(ps[:, :n], wt[:, c, :], hT[:, c, gs],
                                         start=(c == 0), stop=(c == NE - 1),
                                         skip_group_check=True)
                    nc.vector.tensor_scalar_add(
                        out=dst[:, eo, gs], in0=ps[:, :n],
                        scalar1=gb_sb[:, bcol + eo:bcol + eo + 1])
        # V token-major; e_out groups of 512 = 8 heads each
        for tt in range(NCH):
            for g2 in range(2):
                ps = pj.tile([128, 512], F32, tag="ps")
                for c in range(NE):
                    nc.tensor.matmul(ps[:], hT[:, c, tt * 128:(tt + 1) * 128],
                                     wv_all[:, g2 * NE + c, :],
                                     start=(c == 0), stop=(c == NE - 1),
                                     skip_group_check=True)
                nc.vector.tensor_copy(
                    out=v4[:, tt, g2 * 8:(g2 + 1) * 8, 0:64],
                    in_=ps[:].rearrange("p (h d) -> p h d", d=64))

    # ---- Phase 3: attention ---------------------------------------------
    wv_pool.release()

    with ExitStack() as p3:
        pss = p3.enter_context(tc.tile_pool(name="psS", bufs=2, space="PSUM"))
        pso = p3.enter_context(tc.tile_pool(name="psO", bufs=3, space="PSUM"))
        ptp = p3.enter_context(tc.tile_pool(name="pT", bufs=4))
        rcp = p3.enter_context(tc.tile_pool(name="rcp", bufs=4))
        bnc = p3.enter_context(tc.tile_pool(name="bnc", bufs=6, space="DRAM"))

        for qg in range(2):
            qs = QGS[qg]
            pairs = PAIRS_QG[qg]
            npair = len(pairs)
            for h in range(H):
                pt, po = h // 2, (h % 2) * 64
                ps_o = pso.tile([128, 512], F32, tag="o")
                for pi, pair in enumerate(pairs):
                    ps_s = pss.tile([128, MASK_W], F32, tag="s")
                    for k, c in enumerate(pair):
                        nc.tensor.matmul(
                            ps_s[:, k * 512:k * 512 + 512],
                            kT[po:po + 64, pt, c * 128:(c + 1) * 128],
                            qT[po:po + 64, pt, qs],
                            start=True, stop=True, skip_group_check=True)
                    w = len(pair) * 512
                    pT = ptp.tile([128, MASK_W], BF, tag="pT")
                    nc.scalar.activation(pT[:, :w], ps_s[:, :w], AF.Exp)
                    nc.vector.tensor_mul(pT[:, :w], pT[:, :w],
                                         mask_sb[:, MASK_J0[qg] + pi, :w])
                    for k, c in enumerate(pair):
                        nc.tensor.matmul(
                            ps_o[0:65, :],
                            v_sb[:, c, h * 65:h * 65 + 65],
                            pT[:, k * 512:k * 512 + 512],
                            start=(pi == 0 and k == 0),
                            stop=(pi == npair - 1 and k == len(pair) - 1),
                            skip_group_check=True)
                # normalize by the denominator row (ones column of V'):
                # bounce the raw row through DRAM to partition-broadcast it,
                # then a fast approximate reciprocal on the broadcast tile
                # (custom-DVE op requires SBUF input: PSUM reads misbehave).
                sden = rcp.tile([1, 512], F32, tag="sden")
                nc.scalar.copy(sden[:], ps_o[64:65, :])
                dden = bnc.tile([1, 512], F32, tag="dden")
                nc.sync.dma_start(dden[:], sden[:])
                denb = rcp.tile([64, 512], F32, tag="denb")
                nc.sync.dma_start(denb[:], dden[:].partition_broadcast(64))
                nc.vector.reciprocal_approx_fast(out=denb[:], in_=denb[:])
                nc.vector.tensor_mul(attnT[po:po + 64, pt, qs],
                                     ps_o[0:64, :], denb[:])
                # + bv (per-partition in feature-major layout)
                nc.vector.tensor_scalar_add(
                    out=attnT[po:po + 64, pt, qs],
                    in0=attnT[po:po + 64, pt, qs],
                    scalar1=gb_sb[po:po + 64, 48 + pt:48 + pt + 1])

    # ---- Phase 4: Wo + residual -> h2T; LN2 -> g2T ---------------------
    with ExitStack() as p4:
        pj = p4.enter_context(tc.tile_pool(name="pj4", bufs=4, space="PSUM"))
        for eo in range(NE):
            wt = ws2_pool.tile([128, NE, 128], BF, tag="w")
            nc.sync.dma_start(wt[:], wos[eo])
            for qg in range(2):
                qs = QGS[qg]
                ps = pj.tile([128, 512], F32, tag="ps")
                for c in range(NE):
                    nc.tensor.matmul(ps[:], wt[:, c, :], attnT[:, c, qs],
                                     start=(c == 0), stop=(c == NE - 1),
                                     skip_group_check=True)
                nc.vector.scalar_tensor_tensor(
                    h2T[:, eo, qs], ps[:], gb_sb[:, 56 + eo:56 + eo + 1],
                    hT[:, eo, qs], op0=OP.add, op1=OP.add)

    attn_pool.release()
    ws2_pool.release()
    kqv.release()
    hT_pool.release()
    mask_pool.release()

    with ExitStack() as p5:
        _tln(tc, p5, h2T, g2T, gb_sb[:, 16:24], gb_sb[:, 24:32], T, QGS,
             dram, ones, eps_row, "2")

    # ---- Phase 5: MLP + residual -> outT -------------------------------
    with ExitStack() as p6:
        ws = p6.enter_context(tc.tile_pool(name="ws6", bufs=3))
        wp = p6.enter_context(tc.tile_pool(name="wp6", bufs=3))
        pj = p6.enter_context(tc.tile_pool(name="pj6", bufs=4, space="PSUM"))
        up = p6.enter_context(tc.tile_pool(name="uT", bufs=1))
        op_ = p6.enter_context(tc.tile_pool(name="outp", bufs=3))
        uT = up.tile([128, NHT, T], BF, tag="uT")
        for ht in range(NHT):
            wt = ws.tile([128, NE, 128], BF, tag="w")
            nc.sync.dma_start(wt[:], wfcs[ht])
            for qg in range(2):
                qs = QGS[qg]
                ps = pj.tile([128, 512], F32, tag="ps")
                for c in range(NE):
                    nc.tensor.matmul(ps[:], wt[:, c, :], g2T[:, c, qs],
                                     start=(c == 0), stop=(c == NE - 1),
                                     skip_group_check=True)
                if not SIM_SAFE_GELU:
                    nc.scalar.activation(uT[:, ht, qs], ps[:],
                                         AF.Gelu_apprx_tanh,
                                         bias=bfc_sb[:, ht:ht + 1])
                else:
                    # new_gelu(u) = u * sigmoid(2c*(u + 0.044715 u^3))
                    u_sb = ws.tile([128, 512], BF, tag="gelu_u")
                    nc.vector.tensor_scalar_add(
                        out=u_sb[:], in0=ps[:],
                        scalar1=bfc_sb[:, ht:ht + 1])
                    t = ws.tile([128, 512], BF, tag="gelu_t")
                    nc.scalar.activation(t[:], u_sb[:], AF.Square)
                    z = ws.tile([128, 512], BF, tag="gelu_z")
                    nc.vector.scalar_tensor_tensor(
                        z[:], t[:], 0.044715, u_sb[:],
                        op0=OP.mult, op1=OP.mult)
                    nc.vector.tensor_add(z[:], z[:], u_sb[:])
                    g = ws.tile([128, 512], BF, tag="gelu_g")
                    nc.scalar.activation(
                        g[:], z[:], AF.Sigmoid,
                        scale=float(2.0 * np.sqrt(2.0 / np.pi)))
                    nc.vector.tensor_mul(uT[:, ht, qs], u_sb[:], g[:])
        for eo in range(NE):
            wt = wp.tile([128, NHT, 128], BF, tag="w")
            nc.sync.dma_start(wt[:], wprjs[eo])
            for qg in range(2):
                qs = QGS[qg]
                ps = pj.tile([128, 512], F32, tag="ps")
                for c in range(NHT):
                    nc.tensor.matmul(ps[:], wt[:, c, :], uT[:, c, qs],
                                     start=(c == 0), stop=(c == NHT - 1),
                                     skip_group_check=True)
                ot = op_.tile([128, 512], F32, tag="ot")
                nc.vector.scalar_tensor_tensor(
                    ot[:], ps[:], bprj_sb[:, eo:eo + 1], h2T[:, eo, qs],
                    op0=OP.add, op1=OP.add)
                nc.sync.dma_start(outT[eo][:, qs], ot[:])


def _build():
    if "nc" in _CACHE:
        return _CACHE["nc"]
    nc = bacc.Bacc("TRN2", target_bir_lowering=False, debug=False,
                   num_devices=N_CORES)

    def din(name, shape, dt=BF):
        return nc.dram_tensor(name, shape, dt, kind="ExternalInput").ap()

    ins = [
        din("xkv", [NE, 128, SKV]),
        din("wqs", [NE, 128, NE, 128]),
        din("wks", [NE, 128, NE, 128]),
        din("wvs", [2, NE, 128, 512]),
        din("wos", [NE, 128, NE, 128]),
        din("wfcs", [NHT, 128, NE, 128]),
        din("wprjs", [NE, 128, NHT, 128]),
        din("mask", [N_MASK, 128, MASK_W]),
        din("gb", [128, 64], F32),
        din("bfc", [128, 32], F32),
        din("bprj", [128, 8], F32),
    ]
    outT = nc.dram_tensor("outT", [NE, 128, T], F32, kind="ExternalOutput").ap()

    with tile.TileContext(nc) as tc:
        with ExitStack() as ctx:
            _program(tc, ctx, outT, ins)
    nc.compile()
    _CACHE["nc"] = nc
    return nc


# ------------------------------------------------------------------ host code

def _prep_shared(args):
    (ln1_g, ln1_b, ln2_g, ln2_b, wq, bq, wk, bk, wv, bv, wo, bo,
     w_fc, b_fc, w_proj, b_proj) = [np.asarray(a, np.float32) for a in args]
    bf = ml_dtypes.bfloat16

    def eo_layout(wT, nk, nm):  # wT [nk*128, nm*128] -> [nm, 128, nk, 128]
        return np.ascontiguousarray(
            wT.reshape(nk, 128, nm, 128).transpose(2, 1, 0, 3).astype(bf))

    def cols(v, n):  # [n*128] -> [128, n]
        return np.ascontiguousarray(
            np.asarray(v, np.float32).reshape(n, 128).T)

    scale = np.float32(D ** -0.5)
    d = {}
    d["wqs"] = eo_layout((wq * scale).T, NE, NE)
    d["wks"] = eo_layout(wk.T, NE, NE)
    d["wvs"] = np.ascontiguousarray(
        wv.T.reshape(NE, 128, 2, 512).transpose(2, 0, 1, 3).astype(bf))
    d["wos"] = eo_layout(wo.T, NE, NE)
    d["wfcs"] = eo_layout(w_fc.T, NE, NHT)
    d["wprjs"] = eo_layout(w_proj.T, NHT, NE)

    gb = np.zeros((128, 64), np.float32)
    gb[:, 0:8] = cols(ln1_g, 8)
    gb[:, 8:16] = cols(ln1_b, 8)
    gb[:, 16:24] = cols(ln2_g, 8)
    gb[:, 24:32] = cols(ln2_b, 8)
    gb[:, 32:40] = cols(bq * scale, 8)
    gb[:, 40:48] = cols(bk, 8)
    gb[:, 48:56] = cols(bv, 8)
    gb[:, 56:64] = cols(bo, 8)
    d["gb"] = gb
    d["bfc"] = cols(b_fc, 32)
    d["bprj"] = cols(b_proj, 8)
    return d


def _core_inputs(x, mask, shared, core):
    bf = ml_dtypes.bfloat16
    b, half = divmod(core, 2)
    qtok = np.arange(half * T, (half + 1) * T)
    need = np.where(mask[qtok].any(axis=0))[0]
    extra = np.setdiff1d(need, qtok)
    nreal = T + len(extra)
    assert nreal <= SKV, (core, nreal)
    kv = np.concatenate([qtok, extra,
                         np.zeros(SKV - nreal, np.int64)])

    xkvT = x[b].T[:, kv].astype(bf)  # [E, SKV]
    m = np.zeros((T, SKV), np.float32)
    m[:, :nreal] = mask[np.ix_(qtok, kv[:nreal])]
    mT = m.T  # [SKV, T]
    mtiles = np.zeros((N_MASK, 128, MASK_W), np.float32)
    for qg in range(2):
        active = {c for pair in PAIRS_QG[qg] for c in pair}
        skipped = [c for c in range(NCH) if c not in active]
        for c in skipped:
            assert not mT[c * 128:(c + 1) * 128,
                          qg * 512:(qg + 1) * 512].any(), (core, qg, c)
        for pi, pair in enumerate(PAIRS_QG[qg]):
            for k, c in enumerate(pair):
                mtiles[MASK_J0[qg] + pi, :, k * 512:(k + 1) * 512] = \
                    mT[c * 128:(c + 1) * 128, qg * 512:(qg + 1) * 512]

    im = dict(shared)
    im["xkv"] = np.ascontiguousarray(xkvT.reshape(NE, 128, SKV))
    im["mask"] = np.ascontiguousarray(mtiles.astype(bf))
    return im, qtok


def kernel(x, ln1_g, ln1_b, ln2_g, ln2_b, wq, bq, wk, bk, wv, bv, wo, bo,
           w_fc, b_fc, w_proj, b_proj, mask):
    x = np.asarray(x, np.float32)
    mask = np.asarray(mask, bool)

    nc = _build()
    shared = _prep_shared((ln1_g, ln1_b, ln2_g, ln2_b, wq, bq, wk, bk, wv,
                           bv, wo, bo, w_fc, b_fc, w_proj, b_proj))

    in_maps, qtoks = [], []
    for core in range(N_CORES):
        im, qtok = _core_inputs(x, mask, shared, core)
        in_maps.append(im)
        qtoks.append(qtok)

    br = bass_utils.run_bass_kernel_spmd(nc, in_maps,
                                         core_ids=list(range(N_CORES)))
    out = np.empty((B, S, E), np.float32)
    for core in range(N_CORES):
        b, half = divmod(core, 2)
        oT = br.results[core]["outT"]  # [NE, 128, T]
        out[b, qtoks[core], :] = oT.reshape(E, T).T
    return out
